# revision 16
# baseline (speedup 1.0000x reference)
"""Trainium2 Bass kernel: CausalCrossAttention, 8-core SPMD, v3.

Sharding (unchanged from v1/v2): core = (batch b, frame-residue r); frames
t = r + 4*f.  All work core-local, no collectives.

v3 restructure on top of v2's all-bf16 HBM traffic:
  * GroupNorm folded INTO the attention constants (exact algebra):
        scores[s,q] = sum_c kq[c,s] (a_c x[c,q] + b_c)
                    = sum_c (kq*a)[c,s] x[c,q]  +  (sum_c kq[c,s] b_c)
    so per frame we scale kq by the per-channel a (tiny [128,4,64] DVE op)
    and fold the b-term into the per-s additive mask bias.  The [128,4096]
    ScalarE normalize per frame (the h tile) disappears entirely.
  * Softmax along the PARTITION axis (s), scores stay [s=64, q=1024]:
    exp reads PSUM directly with the mask as per-partition ACT bias, the
    denominators come from a ones-row matmul (partition reduce on PE), the
    1/l broadcast across partitions is a K=1 matmul, and the out-projection
    consumes p_hat [s, q] directly -- all four PE transposes per frame and
    both PSUM->SBUF score copies from v1/v2 are gone.
  * Weights ship as two host-concatenated bf16 blobs (k-side first so kq is
    ready early) -> 2 big DMAs instead of 5.
"""

import numpy as np
import ml_dtypes

import concourse.bass as bass
import concourse.bacc as bacc
import concourse.mybir as mybir
import concourse.tile as tile
from concourse.bass_utils import run_bass_kernel_spmd
from concourse.masks import make_identity

B, C, T, H, W = 2, 512, 16, 32, 32
HW = H * W
S, D = 64, 1024
G = 32
CPG = C // G
NCORES = 8
FPC = (B * T) // NCORES
NCH = C // 128
NDCH = D // 128
EPS = 1e-5
SCALE = float(C) ** -0.5
NEGINF = -1e9
MAGIC_HALF = 0x5F3759DF - 0x00400000

F32 = mybir.dt.float32
BF16 = mybir.dt.bfloat16
FP8 = mybir.dt.float8e4
I32 = mybir.dt.int32
BF16NP = ml_dtypes.bfloat16
FP8NP = ml_dtypes.float8_e4m3

Identity = mybir.ActivationFunctionType.Identity
Copy = mybir.ActivationFunctionType.Copy
Exp = mybir.ActivationFunctionType.Exp
Alu = mybir.AluOpType

LAST_RESULT = None
_GRAPH_CACHE = {}


def _build(with_bq: bool, with_bkv: bool, with_bo: bool) -> bass.Bass:
    nc = bacc.Bacc()

    x_d = nc.declare_dram_parameter("x", [128, FPC, NCH, HW], BF16, isOutput=False)
    ctx_d = nc.declare_dram_parameter("ctx", [128, NDCH, S], FP8, isOutput=False)
    wkv_d = nc.declare_dram_parameter("wkv8", [128, 2 * NDCH, C], FP8, isOutput=False)
    wqo_d = nc.declare_dram_parameter("wqo8", [128, 2 * NCH, C], FP8, isOutput=False)
    consts_d = nc.declare_dram_parameter("consts", [128, 20], F32, isOutput=False)
    bq_d = nc.declare_dram_parameter("bqT", [128, NCH], F32, isOutput=False)
    bkv_d = nc.declare_dram_parameter("bkv", [1, 2 * C], F32, isOutput=False)
    bo_d = nc.declare_dram_parameter("bo", [1, C], F32, isOutput=False)
    emat_d = nc.declare_dram_parameter("emat", [8, 128], F32, isOutput=False)
    out_d = nc.declare_dram_parameter("out", [128, FPC, NCH, HW], BF16, isOutput=True)

    with tile.TileContext(nc) as tc:
        with (
            tc.tile_pool(name="consts", bufs=1) as wp,
            tc.tile_pool(name="xp", bufs=4) as xp,
            tc.tile_pool(name="kqp", bufs=2) as kqp,
            tc.tile_pool(name="small", bufs=2) as small,
            tc.tile_pool(name="psO", bufs=2, space="PSUM") as psO,
            tc.tile_pool(name="psS", bufs=1, space="PSUM") as psS,
            tc.tile_pool(name="psT", bufs=1, space="PSUM") as psT,
        ):
            # ---------------- constants ----------------
            # gamma/beta/gmat/mask ship as ONE packed DMA (tiny transfers at
            # the head of a ring each cost ~1us of latency before the weight
            # stream can start)
            consts_sb = wp.tile([128, 20], F32)
            gammaT_sb = consts_sb[:, 0:NCH]
            betaT_sb = consts_sb[:, NCH:2 * NCH]
            gmat_sb = consts_sb[:, 2 * NCH:2 * NCH + 8]
            maskS_sb = consts_sb[0:S, 2 * NCH + 8:2 * NCH + 8 + FPC]
            emat_sb = wp.tile([8, 128], F32)
            maskv_sb = wp.tile([S, FPC], F32)      # SCALE*(mask + b-fold ...)
            identity = wp.tile([128, 128], BF16)
            magic_sb = wp.tile([8, NCH], I32)
            act_scr2 = wp.tile([128, HW], BF16)    # ACT-stats scratch
            dve_scr = wp.tile([128, HW], BF16)     # DVE ttr discard scratch

            make_identity(nc, identity[:])
            nc.gpsimd.memset(magic_sb[:], MAGIC_HALF)

            # HAM warm-up: keep the PE busy from t~0 until the weight DMAs
            # land so the clock-gate opens (1.2 -> 2.4 GHz) before real work.
            warm_ps = psO.tile([S, 128], F32, tag="ps_o")
            for _ in range(64):
                nc.tensor.matmul(
                    warm_ps[:], lhsT=identity[:, :S], rhs=identity[:],
                    start=True, stop=True)

            # ---------------- pipelined x-loads + statistics -------------
            x_tiles = [None] * FPC
            ab_tiles = [None] * FPC
            mv_tiles = [None] * FPC

            def emit_x_load(f, eng=None):
                x_sb = xp.tile([128, NCH, HW], BF16)
                nc.sync.dma_start(out=x_sb[:], in_=x_d[:, f, :, :])
                x_tiles[f] = x_sb

            def ensure_mv(f):
                if mv_tiles[f] is None:
                    mv = small.tile([128, NCH, 2], F32, name=f"mv{f}")
                    mv_tiles[f] = mv
                return mv_tiles[f]

            def emit_stats_dve(f, cis):
                # bn_stats chunks on DVE: mv[:, ci] = (mean, E[x^2])
                x_sb = x_tiles[f]
                mv = ensure_mv(f)
                n = len(cis)
                st6 = small.tile([128, n, 2, 6], F32, name=f"st6_{f}")
                msq = small.tile([128, n], F32, name=f"msq_{f}")
                for i, ci in enumerate(cis):
                    xv = x_sb[:, ci, :].rearrange("p (a b) -> p a b", a=2)
                    for k2 in range(2):
                        nc.vector.bn_stats(out=st6[:, i, k2, :], in_=xv[:, k2, :])
                    nc.vector.bn_aggr(out=mv[:, ci, :], in_=st6[:, i, :, :])
                c0, c1 = cis[0], cis[-1] + 1
                nc.vector.tensor_mul(msq[:], mv[:, c0:c1, 0], mv[:, c0:c1, 0])
                nc.vector.tensor_add(mv[:, c0:c1, 1], mv[:, c0:c1, 1], msq[:])

            def emit_stats_act(f, cis):
                # ACT one-pass accumulations (1/HW folded into the scale)
                x_sb = x_tiles[f]
                mv = ensure_mv(f)
                for ci in cis:
                    nc.scalar.activation(
                        out=act_scr2[:], in_=x_sb[:, ci, :], func=Copy,
                        scale=1.0 / HW, accum_out=mv[:, ci, 0:1])
                    nc.scalar.activation(
                        out=act_scr2[:], in_=x_sb[:, ci, :],
                        func=mybir.ActivationFunctionType.Square,
                        scale=1.0 / 32.0, accum_out=mv[:, ci, 1:2])

            def emit_stats_fold(f):
                psum_g = psO.tile([8, 8], F32, tag="ps_o")
                nc.tensor.matmul(
                    psum_g[:], lhsT=gmat_sb[:],
                    rhs=mv_tiles[f][:].rearrange("p a b -> p (a b)"),
                    start=True, stop=True,
                )
                return psum_g

            def emit_stats_finish(f, psum_g):
                gs = small.tile([8, NCH, 2], F32)
                nc.vector.tensor_copy(
                    out=gs[:], in_=psum_g[:].rearrange("p (a b) -> p a b", a=NCH))
                gsq = small.tile([8, NCH], F32)
                nc.vector.tensor_mul(gsq[:], gs[:, :, 0], gs[:, :, 0])
                hx = small.tile([8, NCH], F32)
                nc.vector.tensor_sub(hx[:], gs[:, :, 1], gsq[:])
                nc.vector.tensor_scalar(
                    out=hx[:], in0=hx[:], scalar1=EPS, scalar2=0.5,
                    op0=Alu.add, op1=Alu.mult)
                ya = small.tile([8, NCH], F32)
                yb = small.tile([8, NCH], F32)
                sh = small.tile([8, NCH], I32)
                nc.vector.tensor_scalar(
                    out=sh[:], in0=hx[:].bitcast(I32), scalar1=1, scalar2=None,
                    op0=Alu.arith_shift_right)
                nc.vector.tensor_sub(ya[:].bitcast(I32), magic_sb[:], sh[:])
                u = small.tile([8, NCH], F32)
                cur, nxt = ya, yb
                for _ in range(2):
                    nc.vector.tensor_mul(u[:], cur[:], cur[:])
                    nc.vector.tensor_mul(u[:], u[:], hx[:])
                    nc.vector.scalar_tensor_tensor(
                        out=nxt[:], in0=u[:], scalar=1.5, in1=cur[:],
                        op0=Alu.subtract, op1=Alu.mult)
                    cur, nxt = nxt, cur
                nc.vector.tensor_copy(out=gs[:, :, 1], in_=cur[:])
                psum_e = psO.tile([128, NCH, 2], F32, tag="ps_o")
                nc.tensor.matmul(
                    psum_e[:].rearrange("p a b -> p (a b)"),
                    lhsT=emat_sb[:], rhs=gs[:].rearrange("p a b -> p (a b)"),
                    start=True, stop=True,
                )
                a_sb = small.tile([128, NCH], F32)
                t_sb = small.tile([128, NCH], F32)
                bb_sb = small.tile([128, NCH], BF16)
                nc.vector.tensor_mul(a_sb[:], psum_e[:, :, 1], gammaT_sb[:])
                nc.vector.tensor_mul(t_sb[:], psum_e[:, :, 0], a_sb[:])
                # b in bf16 directly (it only feeds the tiny b-fold matmul)
                nc.vector.tensor_sub(bb_sb[:], betaT_sb[:], t_sb[:])
                ab_tiles[f] = (a_sb, bb_sb)

            # ---- weights at rest in fp8: kv-proj runs as fp8 matmuls, and
            # wq/wo are upcast to bf16 on the (idle) ScalarE on arrival ----
            ctx_f8 = wp.tile([128, NDCH, S], FP8)
            wkv_f8 = wp.tile([128, 2 * NDCH, C], FP8)
            wqo_f8 = wp.tile([128, 2 * NCH, C], FP8)
            wq_bf = wp.tile([128, NCH, C], BF16)
            wo_bf = wp.tile([128, NCH, C], BF16)

            # ALL input DMAs ride the Sync-engine HWDGE ring: a dma_start
            # occupies its issuing engine's queue (and stalls it when the
            # ring is full), and SP has nothing else to do.  Output DMAs go
            # through GpSimd's SWDGE for the same reason.  One queue drives
            # all 16 SDMA engines, so this costs no bandwidth.
            nc.sync.dma_start(out=ctx_f8[:], in_=ctx_d[:, :, :])
            nc.sync.dma_start(out=wkv_f8[:, :NDCH, :], in_=wkv_d[:, :NDCH, :])
            nc.sync.dma_start(out=wqo_f8[:], in_=wqo_d[:, :, :])
            emit_x_load(0)
            nc.sync.dma_start(out=wkv_f8[:, NDCH:, :], in_=wkv_d[:, NDCH:, :])
            nc.sync.dma_start(out=consts_sb[:], in_=consts_d[:, :])
            nc.sync.dma_start(out=emat_sb[:], in_=emat_d[:, :])
            emit_x_load(1)
            emit_x_load(2)
            emit_x_load(3)
            emit_stats_dve(0, [0, 1, 2, 3])
            nc.scalar.activation(out=wq_bf[:], in_=wqo_f8[:, :NCH, :], func=Copy)
            nc.scalar.activation(out=wo_bf[:], in_=wqo_f8[:, NCH:, :], func=Copy)
            emit_stats_dve(1, [0, 1, 2, 3])

            if with_bkv:
                ones64r = wp.tile([1, S], BF16)
                nc.vector.memset(ones64r[:], 1.0)
                stb = small.tile([1, 2 * C], F32)
                nc.sync.dma_start(out=stb[:], in_=bkv_d[:, :])
                bkv_bf = wp.tile([1, 2 * C], BF16)
                nc.vector.tensor_copy(out=bkv_bf[:], in_=stb[:])
            if with_bq:
                bqT_sb = wp.tile([128, NCH], F32)
                nc.sync.dma_start(out=bqT_sb[:], in_=bq_d[:, :])
            if with_bo:
                ones1024 = wp.tile([1, HW], BF16)
                nc.vector.memset(ones1024[:], 1.0)
                sbo = small.tile([1, C], F32)
                nc.sync.dma_start(out=sbo[:], in_=bo_d[:, :])
                bo_bf = wp.tile([1, C], BF16)
                nc.vector.tensor_copy(out=bo_bf[:], in_=sbo[:])

            # ------------- context constants: k (transposed), kq, vo ----------
            kT_sb = wp.tile([128, NCH, S], BF16)
            vo_bf = wp.tile([S, C], BF16)
            # k = ctx @ WkT  -> [S, C], then transpose to [c, s] chunks
            psum_kv = psO.tile([S, C], F32, tag="ps_o")
            for dci in range(NDCH):
                nc.tensor.matmul(
                    psum_kv[:], lhsT=ctx_f8[:, dci, :], rhs=wkv_f8[:, dci, :],
                    start=(dci == 0), stop=(dci == NDCH - 1 and not with_bkv),
                )
            if with_bkv:
                nc.tensor.matmul(
                    psum_kv[:], lhsT=ones64r[:], rhs=bkv_bf[:, :C],
                    start=False, stop=True)
            k_sb = small.tile([S, C], BF16, tag="st_kvsb")
            nc.scalar.activation(out=k_sb[:], in_=psum_kv[:], func=Copy)
            psum_t = psO.tile([128, NCH, S], BF16, tag="ps_o")
            for ci in range(NCH):
                nc.tensor.transpose(
                    psum_t[:, ci, :], k_sb[:, ci * 128:(ci + 1) * 128],
                    identity[:64, :64])
            nc.scalar.activation(out=kT_sb[:], in_=psum_t[:], func=Copy)

            # kq^T[c, s] = sum_c' wq[c', c] k[s, c']
            kqT_sb = wp.tile([128, NCH, S], BF16)
            psum_kq = psO.tile([128, NCH, S], F32, tag="ps_o")
            for co in range(NCH):
                for ci in range(NCH):
                    nc.tensor.matmul(
                        psum_kq[:, co, :],
                        lhsT=wq_bf[:, ci, co * 128:(co + 1) * 128],
                        rhs=kT_sb[:, ci, :],
                        start=(ci == 0), stop=(ci == NCH - 1),
                    )
            nc.scalar.activation(out=kqT_sb[:], in_=psum_kq[:], func=Copy)

            # v = ctx @ WvT -> [S, C];  vo[s, oc] = sum_c v[s, c] wo[oc, c]
            # (v^T not needed: vo = v @ woT computed via vT chunks)
            psum_v = psO.tile([S, C], F32, tag="ps_o")
            for dci in range(NDCH):
                nc.tensor.matmul(
                    psum_v[:], lhsT=ctx_f8[:, dci, :], rhs=wkv_f8[:, NDCH + dci, :],
                    start=(dci == 0), stop=(dci == NDCH - 1 and not with_bkv),
                )
            if with_bkv:
                nc.tensor.matmul(
                    psum_v[:], lhsT=ones64r[:], rhs=bkv_bf[:, C:],
                    start=False, stop=True)
            v_sb = small.tile([S, C], BF16, tag="st_kvsb")
            nc.scalar.activation(out=v_sb[:], in_=psum_v[:], func=Copy)
            psum_vt = psO.tile([128, NCH, S], BF16, tag="ps_o")
            for ci in range(NCH):
                nc.tensor.transpose(
                    psum_vt[:, ci, :], v_sb[:, ci * 128:(ci + 1) * 128],
                    identity[:64, :64])
            vT_sb = wp.tile([128, NCH, S], BF16)
            nc.scalar.activation(out=vT_sb[:], in_=psum_vt[:], func=Copy)
            psum_vo = psO.tile([S, C], F32, tag="ps_o")
            for ci in range(NCH):
                nc.tensor.matmul(
                    psum_vo[:], lhsT=vT_sb[:, ci, :], rhs=wo_bf[:, ci, :],
                    start=(ci == 0), stop=(ci == NCH - 1),
                )
            nc.scalar.activation(out=vo_bf[:], in_=psum_vo[:], func=Copy)

            # bq folds into the (pre-scaled) mask template before any
            # per-frame maskv column is derived from it
            if with_bq:
                bq_bf = wp.tile([128, NCH], BF16)
                nc.vector.tensor_copy(out=bq_bf[:], in_=bqT_sb[:])
                psum_bq = psO.tile([S, 1], F32, tag="ps_o")
                for ci in range(NCH):
                    nc.tensor.matmul(
                        psum_bq[:], lhsT=kT_sb[:, ci, :], rhs=bq_bf[:, ci:ci + 1],
                        start=(ci == 0), stop=(ci == NCH - 1),
                    )
                nc.vector.scalar_tensor_tensor(
                    out=maskS_sb[:], in0=psum_bq[:].to_broadcast((S, FPC)),
                    scalar=SCALE, in1=maskS_sb[:], op0=Alu.mult, op1=Alu.add)

            # per-frame stats chain: group fold -> rsqrt finish -> b-fold ->
            # maskv column.  Emitted per frame (frames 2/3 from inside the
            # loop) so frame 0 never waits on frame 3's statistics.
            def emit_chain(f):
                pg = emit_stats_fold(f)
                emit_stats_finish(f, pg)
                _, bb_sb = ab_tiles[f]
                psum_mb = psO.tile([S, 1], F32, tag="ps_o")
                for ci in range(NCH):
                    nc.tensor.matmul(
                        psum_mb[:], lhsT=kqT_sb[:, ci, :],
                        rhs=bb_sb[:, ci:ci + 1],
                        start=(ci == 0), stop=(ci == NCH - 1),
                    )
                nc.vector.scalar_tensor_tensor(
                    out=maskv_sb[:, f:f + 1], in0=psum_mb[:], scalar=SCALE,
                    in1=maskS_sb[:, f:f + 1], op0=Alu.mult, op1=Alu.add)

            emit_chain(0)
            emit_chain(1)

            # ------------- software-pipelined frame loop -------------
            # Engine queues are strictly in-order, so emission order IS the
            # schedule: frame f+1's score matmuls are emitted before frame
            # f's softmax transposes, and frame f-1's output projection
            # lands between the two transpose groups.  The PE then never
            # sits parked behind a cross-engine dependency it could outrun,
            # which also keeps the HAM clock-gate open.

            def emit_back(bf_, bx_sb, bpn_flat):
                for oc in range(NCH):
                    # oc 0/1: x folds into PSUM via an identity matmul and
                    # ScalarE does the PSUM->SBUF move; oc 2/3: DVE adds.
                    on_act = oc < 2
                    psum_o = psO.tile([128, 2, 512], F32, tag="ps_o")
                    for half in range(2):
                        if on_act:
                            nc.tensor.matmul(
                                psum_o[:, half, :], lhsT=identity[:],
                                rhs=bx_sb[:, oc, half * 512:(half + 1) * 512],
                                start=True, stop=False)
                        nc.tensor.matmul(
                            psum_o[:, half, :],
                            lhsT=vo_bf[:, oc * 128:(oc + 1) * 128],
                            rhs=bpn_flat[:, half * 512:(half + 1) * 512],
                            start=not on_act, stop=not with_bo)
                        if with_bo:
                            nc.tensor.matmul(
                                psum_o[:, half, :],
                                lhsT=bo_bf[:, oc * 128:(oc + 1) * 128],
                                rhs=ones1024[:, :512], start=False, stop=True)
                    if on_act:
                        nc.scalar.activation(
                            out=bx_sb[:, oc, :],
                            in_=psum_o[:].rearrange("p a b -> p (a b)"),
                            func=Copy)
                    else:
                        nc.vector.tensor_add(
                            bx_sb[:, oc, :],
                            psum_o[:].rearrange("p a b -> p (a b)"),
                            bx_sb[:, oc, :])
                    if bf_ == FPC - 1:
                        # last frame: per-chunk DMAs shave the serial tail,
                        # on the fast (and by now idle) Sync HWDGE ring
                        nc.sync.dma_start(
                            out=out_d[:, bf_, oc:oc + 1, :],
                            in_=bx_sb[:, oc:oc + 1, :])
                if bf_ < FPC - 1:
                    nc.gpsimd.dma_start(
                        out=out_d[:, bf_, :, :], in_=bx_sb[:])

            sc_tiles = [None] * FPC

            def emit_scores(f):
                a_sb, _ = ab_tiles[f]
                kqa_sb = kqp.tile([128, NCH, S], BF16, name=f"kqa{f}")
                nc.vector.tensor_mul(
                    kqa_sb[:], kqT_sb[:],
                    a_sb[:].rearrange("p (a o) -> p a o", o=1)
                        .to_broadcast((128, NCH, S)))
                x_sb = x_tiles[f]
                scs = []
                for half in range(2):
                    psum_sc = psS.tile([S, 512], F32, tag="ps_sc",
                                       name=f"sc{f}h{half}")
                    for ci in range(NCH):
                        nc.tensor.matmul(
                            psum_sc[:],
                            lhsT=kqa_sb[:, ci, :],
                            rhs=x_sb[:, ci, half * 512:(half + 1) * 512],
                            start=(ci == 0), stop=(ci == NCH - 1),
                        )
                    scs.append(psum_sc)
                sc_tiles[f] = scs

            emit_scores(0)
            prev = [None]
            for f in range(FPC):
                x_sb = x_tiles[f]

                # p = exp(SCALE*scores + maskv) straight out of PSUM, per half
                p_bf = small.tile([S, 2, 512], BF16, name=f"p{f}")
                for half in range(2):
                    nc.scalar.activation(
                        out=p_bf[:, half, :], in_=sc_tiles[f][half][:],
                        func=Exp, bias=maskv_sb[:, f:f + 1], scale=SCALE)
                p_flat = p_bf[:].rearrange("p a b -> p (a b)")

                # next frame's scores go into the PE queue first
                if f + 1 < FPC:
                    emit_scores(f + 1)

                # transpose p to q-partition layout for the denominators
                psum_pT = psT.tile([128, 8, S], BF16, tag="ps_x",
                                   name=f"pT{f}")
                for j in range(8):
                    nc.tensor.transpose(
                        psum_pT[:, j, :], p_flat[:, j * 128:(j + 1) * 128],
                        identity[:S, :S])

                # previous frame's output projection fills the PE while the
                # DVE handles this frame's reductions
                if prev[0] is not None:
                    emit_back(*prev[0])
                    prev[0] = None
                if f < 2:
                    emit_stats_act(f + 2, [2, 3])

                l_col = small.tile([128, 8, 1], BF16, name=f"l{f}")
                linv = small.tile([128, 8, 1], BF16, name=f"li{f}")
                with nc.allow_low_precision(reason="softmax denominators "
                                            "only need ~3 digits"):
                    nc.vector.reduce_sum(l_col[:], psum_pT[:],
                                         axis=mybir.AxisListType.X)
                    nc.vector.reciprocal(linv[:], l_col[:])
                pnT_bf = small.tile([128, 8, S], BF16, name=f"pnT{f}")
                nc.vector.tensor_mul(pnT_bf[:], psum_pT[:],
                                     linv[:].to_broadcast((128, 8, S)))
                psum_pn = psT.tile([S, 8, 128], BF16, tag="ps_x",
                                   name=f"pn{f}")
                for j in range(8):
                    nc.tensor.transpose(
                        psum_pn[:, j, :], pnT_bf[:, j, :], identity[:])
                pn_bf = small.tile([S, 8, 128], BF16, name=f"pnb{f}")
                nc.vector.tensor_copy(out=pn_bf[:], in_=psum_pn[:])

                if f < 2:
                    emit_stats_dve(f + 2, [0, 1])
                    emit_chain(f + 2)

                prev[0] = (f, x_sb, pn_bf[:].rearrange("p a b -> p (a b)"))

            emit_back(*prev[0])

    nc.finalize()
    return nc


def _prep_in_maps(x, context, gamma, beta, wq, bq, wkv, bkv, wo, bo):
    f32 = lambda a: np.ascontiguousarray(np.asarray(a, dtype=np.float32))
    x = np.asarray(x, np.float32)
    pm = lambda a, n: a.reshape(n, 128, a.shape[-1]).transpose(1, 0, 2)
    flat = lambda a: a.reshape(128, -1)

    wq_c = pm(np.asarray(wq, np.float32), NCH)                 # [128, 4, C]
    wkvT = np.ascontiguousarray(np.asarray(wkv, np.float32).T)  # [D, 2C]
    wkvk_c = pm(np.ascontiguousarray(wkvT[:, :C]), NDCH)
    wkvv_c = pm(np.ascontiguousarray(wkvT[:, C:]), NDCH)
    woT_c = pm(np.ascontiguousarray(np.asarray(wo, np.float32).T), NCH)
    bqT_c = f32(np.asarray(bq, np.float32).reshape(NCH, 128).T)
    bkv_c = f32(np.asarray(bkv, np.float32).reshape(1, 2 * C))
    gammaT = f32(np.asarray(gamma, np.float32).reshape(NCH, 128).T)
    betaT = f32(np.asarray(beta, np.float32).reshape(NCH, 128).T)
    bo_r = f32(np.asarray(bo, np.float32).reshape(1, C))

    wkv8 = np.ascontiguousarray(
        np.concatenate([wkvk_c, wkvv_c], axis=1)).astype(FP8NP)
    wqo8 = np.ascontiguousarray(
        np.concatenate([wq_c, woT_c], axis=1)).astype(FP8NP)

    gmat = np.zeros((128, 8), np.float32)
    gmat[np.arange(128), np.arange(128) // CPG] = 1.0 / CPG
    emat = np.zeros((8, 128), np.float32)
    emat[np.arange(128) // CPG, np.arange(128)] = 1.0

    in_maps = []
    for core in range(NCORES):
        b, r = divmod(core, 4)
        xs = np.ascontiguousarray(
            x[b, :, r::4, :, :].reshape(NCH, 128, FPC, HW).transpose(1, 2, 0, 3)
        ).astype(BF16NP)
        ctxT = np.ascontiguousarray(
            pm(np.ascontiguousarray(context[b].T.astype(np.float32)), NDCH)
        ).astype(FP8NP)
        mask = np.zeros((S, FPC), np.float32)
        for f in range(FPC):
            t = 4 * f + r
            lim = min(4 * (t + 1), S)
            mask[lim:, f] = NEGINF * SCALE
        consts = np.zeros((128, 20), np.float32)
        consts[:, 0:NCH] = gammaT
        consts[:, NCH:2 * NCH] = betaT
        consts[:, 2 * NCH:2 * NCH + 8] = gmat
        consts[0:S, 2 * NCH + 8:] = mask
        in_maps.append(dict(
            x=xs, ctx=ctxT, wkv8=wkv8, wqo8=wqo8,
            bqT=bqT_c, bkv=bkv_c, bo=bo_r,
            consts=consts, emat=emat,
        ))
    return in_maps


def kernel(x, context, gamma, beta, wq, bq, wkv, bkv, wo, bo,
           _trace=False, **_trace_kwargs):
    global LAST_RESULT
    with_bq = bool(np.any(np.asarray(bq)))
    with_bkv = bool(np.any(np.asarray(bkv)))
    with_bo = bool(np.any(np.asarray(bo)))
    key = (with_bq, with_bkv, with_bo)
    if key not in _GRAPH_CACHE:
        _GRAPH_CACHE[key] = _build(*key)
    nc = _GRAPH_CACHE[key]

    in_maps = _prep_in_maps(x, context, gamma, beta, wq, bq, wkv, bkv, wo, bo)
    res = run_bass_kernel_spmd(nc, in_maps, core_ids=list(range(NCORES)),
                               trace=_trace, **_trace_kwargs)
    LAST_RESULT = res

    out = np.empty((B, C, T, H, W), np.float32)
    for core in range(NCORES):
        b, r = divmod(core, 4)
        out[b, :, r::4, :, :] = res.results[core]["out"].astype(
            np.float32).transpose(2, 0, 1, 3).reshape(C, FPC, H, W)
    return out
